# revision 4
# baseline (speedup 1.0000x reference)
"""Trainium2 Bass kernel: batched complex-waveform similarity.

Math: reference computes
    bank = ifft_ortho(freq)                # [T, L] complex
    score = rx @ conj(bank).T              # [B, T] complex
    sim   = (score.re^2 + score.im^2) / temperature

Since the ortho DFT is unitary,  score = fft_ortho(rx) @ conj(freq).T.
So the kernel never builds the bank: it DFTs rx via a 128x128 bf16
matmul, then runs one big complex GEMM [B,L]x[L,T] in bf16 with fp32
PSUM accumulation, and a fused squared-magnitude epilogue.

Sharding: data-parallel over the rx batch dim across 8 NeuronCores;
freq (as a transposed bf16 [L, T] pair) is replicated on every core.

Per-core schedule (all engines in-order; Tile inserts the sems):
  PE   : 32 warmup matmuls (clock ramp) -> DFT chunk0 -> main n-outer/
         m-inner GEMM stream with DFT chunk1 interleaved after (n0,m1)
  ACT  : -rxf_r casts + t2 = Square(Si)           (PSUM -> SBUF)
  DVE  : rxf casts + out = (Sr^2 + t2) * (1/temp) (custom fused DVE op)
  SP/ACT rings : inputs split so the first-needed tiles land first;
         output DMAs alternate 50/50 across both HWDGE rings
  Pool : SWDGE DMAs for the first rx chunk (starts ~2us before the
         HWDGE rings come up)
"""

import numpy as np
import ml_dtypes

B = 8192
T = 8192
L = 128
NCORES = 8
BPC = B // NCORES  # batch rows per core

_BF16 = ml_dtypes.bfloat16

_CACHE = {}


# --------------------------------------------------------------------------- #
# Custom DVE op: out = (Src0^2 + Src1) * C0
# (Src0 = Sr from PSUM, Src1 = Si^2 staged by ScalarE, C0 = 1/temperature)
# --------------------------------------------------------------------------- #
def _get_sqadd_op():
    import concourse.dve_ops as dve_ops
    from concourse.dve_spec import Spec, Src0, Src1, C0, sq, lower, _has_src1
    from concourse.dve_uop import DveOpSpec

    name = "SQ_ADD_SCALE_ANT"
    for op in dve_ops.OPS:
        if op.name == name:
            return op

    spec = Spec(
        body=(sq(Src0) + Src1) * C0,
        reference=lambda in0, in1, s0, s1, imm2: (
            (in0.astype(np.float32) ** 2 + in1.astype(np.float32)) * s0
        ).astype(np.float32),
    )
    opcode = dve_ops._CUSTOM_DVE_ROW_BASE + len(dve_ops.OPS)
    assert opcode < 0x20
    shas = {}
    for ver in ("v3", "v4"):
        compiled = DveOpSpec(
            name=name, opcode=opcode, uops=lower(spec, ver=ver), rd1_en=_has_src1(spec)
        )
        shas[ver] = compiled.sha(ver)
    op = dve_ops.DveOp(name, spec, subdim=False, uops_sha=shas)
    dve_ops.OPS.append(op)
    dve_ops.CUSTOM_DVE_SPECS[name] = spec
    dve_ops._SUB_OPCODE_FOR_NAME[name] = opcode
    return op


# --------------------------------------------------------------------------- #
# Bass program (one SPMD NeuronCore)
# --------------------------------------------------------------------------- #
def build_nc(bpc=BPC, t=T, debug=False):
    from contextlib import ExitStack

    import concourse.bacc as bacc
    import concourse.bass as bass
    import concourse.mybir as mybir
    import concourse.tile as tile

    f32 = mybir.dt.float32
    bf16 = mybir.dt.bfloat16
    sqadd = _get_sqadd_op()

    NG = 512   # output columns per PSUM group (1 bank)
    FG = 1024  # freq columns per SBUF tile / DMA
    HC = 512   # rx/DFT column chunk
    assert bpc % HC == 0 and t % FG == 0
    n_chunks = bpc // HC
    n_groups = t // NG
    m_tiles = bpc // 128

    nc = bacc.Bacc("TRN2", target_bir_lowering=False, debug=debug, num_devices=NCORES)

    rxt_r = nc.dram_tensor("rxt_r", [L, bpc], bf16, kind="ExternalInput")
    rxt_i = nc.dram_tensor("rxt_i", [L, bpc], bf16, kind="ExternalInput")
    fqt_r = nc.dram_tensor("fqt_r", [L, t], bf16, kind="ExternalInput")
    fqt_i = nc.dram_tensor("fqt_i", [L, t], bf16, kind="ExternalInput")
    w_r = nc.dram_tensor("w_r", [L, L], bf16, kind="ExternalInput")
    w_i = nc.dram_tensor("w_i", [L, L], bf16, kind="ExternalInput")
    w_ni = nc.dram_tensor("w_ni", [L, L], bf16, kind="ExternalInput")
    temp = nc.dram_tensor("temp", [128, 1], f32, kind="ExternalInput")
    out = nc.dram_tensor("out", [bpc, t], f32, kind="ExternalOutput")

    with tile.TileContext(nc) as tc, ExitStack() as ctx:
        consts = ctx.enter_context(tc.tile_pool(name="consts", bufs=1))
        psum = ctx.enter_context(
            tc.tile_pool(name="psum", bufs=4, space=bass.MemorySpace.PSUM)
        )
        sq_pool = ctx.enter_context(tc.tile_pool(name="sq", bufs=6))
        out_pool = ctx.enter_context(tc.tile_pool(name="ob", bufs=16))

        # ---- PE warmup ------------------------------------------------ #
        # Dependency-free matmuls run during the input-DMA window so the
        # HAM clock gate is already ramped when the real matmuls start.
        # Sized to end just as the first rx chunk + W land (~10.8us).
        warm_w = consts.tile([128, 128], bf16)
        nc.gpsimd.memset(warm_w[:], 0)
        warm_ps = psum.tile([128, NG], mybir.dt.float32, tag="si")
        for _ in range(32):
            nc.tensor.matmul(warm_ps[:, 0:128], warm_w[:], warm_w[:], start=True, stop=True)

        # ---- load inputs ---------------------------------------------- #
        # First rx chunk rides the Pool SWDGE ring, which starts issuing
        # ~2us before the HWDGE rings finish their launch latency.
        rx_r = consts.tile([L, bpc], bf16)
        rx_i = consts.tile([L, bpc], bf16)
        c0 = slice(0, HC)
        nc.gpsimd.dma_start(rx_r[:, c0], rxt_r[:, c0])
        nc.gpsimd.dma_start(rx_i[:, c0], rxt_i[:, c0])

        # Sync (SP) ring: everything the first few ms of compute needs,
        # in need-order. Scalar ring (which opens later, behind the ACT
        # table load): the bulk freq tail.
        wr_sb = consts.tile([L, L], bf16)
        nc.sync.dma_start(wr_sb[:], w_r[:, :])
        wni_sb = consts.tile([L, L], bf16)
        nc.sync.dma_start(wni_sb[:], w_ni[:, :])
        wi_sb = consts.tile([L, L], bf16)
        nc.sync.dma_start(wi_sb[:], w_i[:, :])

        fr_sb = []
        fi_sb = []
        for g in range(t // FG):
            fr_sb.append(consts.tile([L, FG], bf16, tag=f"fr{g}", name=f"fr{g}"))
            fi_sb.append(consts.tile([L, FG], bf16, tag=f"fi{g}", name=f"fi{g}"))

        def load_freq(g, eng):
            gs = slice(g * FG, (g + 1) * FG)
            eng.dma_start(fr_sb[g][:], fqt_r[:, gs])
            eng.dma_start(fi_sb[g][:], fqt_i[:, gs])

        load_freq(0, nc.sync)
        for c in range(1, n_chunks):
            cs = slice(c * HC, (c + 1) * HC)
            nc.sync.dma_start(rx_r[:, cs], rxt_r[:, cs])
            nc.sync.dma_start(rx_i[:, cs], rxt_i[:, cs])
        temp_sb = consts.tile([128, 1], f32)
        nc.sync.dma_start(temp_sb[:], temp[:, :])
        load_freq(1, nc.sync)
        for g in range(2, t // FG):
            load_freq(g, nc.scalar)

        # ---- DFT of rx (bf16): rxfT = W @ rxT ------------------------- #
        # W symmetric, so PE's lhsT is W itself.
        # rxfT_r = Wr@rxT_r - Wi@rxT_i ; rxfT_i = Wr@rxT_i + Wi@rxT_r
        rxf_r = consts.tile([L, bpc], bf16)
        rxf_i = consts.tile([L, bpc], bf16)
        rxf_nr = consts.tile([L, bpc], bf16)  # -rxfT_r

        def emit_dft(c):
            cs = slice(c * HC, (c + 1) * HC)
            pr = psum.tile([128, NG], mybir.dt.float32, tag="sr")
            nc.tensor.matmul(pr[:, 0:HC], wr_sb[:], rx_r[:, cs], start=True, stop=False)
            nc.tensor.matmul(pr[:, 0:HC], wni_sb[:], rx_i[:, cs], start=False, stop=True)
            pi = psum.tile([128, NG], mybir.dt.float32, tag="si")
            nc.tensor.matmul(pi[:, 0:HC], wr_sb[:], rx_i[:, cs], start=True, stop=False)
            nc.tensor.matmul(pi[:, 0:HC], wi_sb[:], rx_r[:, cs], start=False, stop=True)
            return pr, pi

        def emit_casts(c, pr, pi, step):
            # DVE: +rxf_r, rxf_i ; ACT: -rxf_r. Chunked so the first main
            # matmuls only gate on their own 128-col slice.
            for k0 in range(0, HC, step):
                ks = slice(c * HC + k0, c * HC + k0 + step)
                kp = slice(k0, k0 + step)
                nc.vector.tensor_copy(rxf_r[:, ks], pr[:, kp])
                nc.vector.tensor_copy(rxf_i[:, ks], pi[:, kp])
                nc.scalar.mul(rxf_nr[:, ks], pr[:, kp], -1.0)

        pr0, pi0 = emit_dft(0)
        emit_casts(0, pr0, pi0, 256)

        invt_sb = consts.tile([128, 1], f32)
        nc.vector.reciprocal(invt_sb[:], temp_sb[:])

        # ---- main complex GEMM + fused |.|^2 epilogue ----------------- #
        # Sr = rxf_r.T @ fr + rxf_i.T @ fi
        # Si = rxf_i.T @ fr - rxf_r.T @ fi
        # n outer so freq group g is first needed ~7us * g into the GEMM
        # stream (the freq loads can't all land before the stream starts).
        ndma = 0
        for n in range(n_groups):
            g, j = divmod(n, FG // NG)
            js = slice(j * NG, (j + 1) * NG)
            for m in range(m_tiles):
                if n == 0 and m == 2 and n_chunks > 1:
                    # second DFT chunk slots in while chunk-0 rows compute
                    pr1, pi1 = emit_dft(1)
                    emit_casts(1, pr1, pi1, 256)
                ms = slice(m * 128, (m + 1) * 128)
                sr = psum.tile([128, NG], mybir.dt.float32, tag="sr")
                si = psum.tile([128, NG], mybir.dt.float32, tag="si")
                nc.tensor.matmul(sr[:], rxf_r[:, ms], fr_sb[g][:, js], start=True, stop=False)
                nc.tensor.matmul(sr[:], rxf_i[:, ms], fi_sb[g][:, js], start=False, stop=True)
                nc.tensor.matmul(si[:], rxf_i[:, ms], fr_sb[g][:, js], start=True, stop=False)
                nc.tensor.matmul(si[:], rxf_nr[:, ms], fi_sb[g][:, js], start=False, stop=True)
                t2 = sq_pool.tile([128, NG], f32)
                nc.scalar.square(t2[:], si[:])
                ob = out_pool.tile([128, NG], f32)
                nc.vector._custom_dve(
                    sqadd, out=ob[:], in0=sr[:], in1=t2[:], s0=invt_sb[:]
                )
                oeng = nc.sync if ndma % 2 == 0 else nc.scalar
                ndma += 1
                oeng.dma_start(out[ms, n * NG : (n + 1) * NG], ob[:])

    nc.compile()
    return nc


def _host_prep(rx_real, rx_imag, freq_real, freq_imag, temperature, bpc=BPC, t=T):
    """Layout marshaling only: shard/transpose/cast inputs for the cores."""
    lk = np.outer(np.arange(L), np.arange(L)).astype(np.float64)
    w = np.exp(-2j * np.pi * lk / L) / np.sqrt(L)  # ortho DFT matrix (symmetric)
    w_r = np.ascontiguousarray(w.real.astype(np.float32).astype(_BF16))
    w_i = np.ascontiguousarray(w.imag.astype(np.float32).astype(_BF16))
    w_ni = np.ascontiguousarray(-w.imag.astype(np.float32)).astype(_BF16)

    fqt_r = np.ascontiguousarray(freq_real[:t].T.astype(_BF16))  # [L, T]
    fqt_i = np.ascontiguousarray(freq_imag[:t].T.astype(_BF16))
    temp_col = np.full((128, 1), np.asarray(temperature), np.float32)

    rxt_r = np.asarray(rx_real, np.float32).T.astype(_BF16)  # [L, B]
    rxt_i = np.asarray(rx_imag, np.float32).T.astype(_BF16)

    in_maps = []
    for c in range(NCORES):
        cs = slice(c * bpc, (c + 1) * bpc)
        in_maps.append(
            {
                "rxt_r": np.ascontiguousarray(rxt_r[:, cs]),
                "rxt_i": np.ascontiguousarray(rxt_i[:, cs]),
                "fqt_r": fqt_r,
                "fqt_i": fqt_i,
                "w_r": w_r,
                "w_i": w_i,
                "w_ni": w_ni,
                "temp": temp_col,
            }
        )
    return in_maps


def kernel(rx_real, rx_imag, freq_real, freq_imag, temperature):
    from concourse.bass_utils import run_bass_kernel_spmd

    if "nc" not in _CACHE:
        _CACHE["nc"] = build_nc()
    nc = _CACHE["nc"]

    in_maps = _host_prep(rx_real, rx_imag, freq_real, freq_imag, temperature)
    res = run_bass_kernel_spmd(nc, in_maps, core_ids=list(range(NCORES)))
    _CACHE["last_result"] = res
    return np.concatenate([r["out"] for r in res.results], axis=0)
